# revision 5
# baseline (speedup 1.0000x reference)
import numpy as np

import concourse.bacc as bacc
import concourse.tile as tile
from concourse import mybir

# Problem: NIMSCrossEntropyLoss
#   preds (4, 4, 4, 512, 512) f32, targets (4, 4, 512, 512) int
#   loss = sum over pixels of [logsumexp_c(preds[:, -1]) - preds[:, -1][target]] / N
# Only the S=-1 slice contributes. Shard the 4*512*512 = 1048576 pixels over
# 8 cores: 131072 pixels/core laid out [128 partitions, 1024 pixels, 4 channels].

N_CORES = 8
P = 128           # partitions
C = 4             # classes
N_BATCH = 4       # reference divides by this

F32 = mybir.dt.float32


def build_nc(f_total, nchunk, finalize=True):
    """Build the Bass module for one core's shard: [P, f_total, C] preds."""
    fc = f_total // nchunk
    assert fc * nchunk == f_total

    nc = bacc.Bacc("TRN2", target_bir_lowering=False, debug=False)
    preds = nc.dram_tensor("preds", (P, f_total, C), F32, kind="ExternalInput").ap()
    tgt = nc.dram_tensor("tgt", (P, f_total), F32, kind="ExternalInput").ap()
    out = nc.dram_tensor("out", (P, 2), F32, kind="ExternalOutput").ap()

    with tile.TileContext(nc) as tc:
        with (
            tc.tile_pool(name="io", bufs=3) as io_pool,
            tc.tile_pool(name="work", bufs=3) as work,
            tc.tile_pool(name="accs", bufs=1) as accs,
        ):
            lse_acc = accs.tile([P, nchunk], F32)
            pt_acc = accs.tile([P, nchunk * C], F32)
            res = accs.tile([P, 2], F32)

            for j in range(nchunk):
                p_tile = io_pool.tile([P, fc, C], F32)
                nc.sync.dma_start(out=p_tile, in_=preds[:, j * fc:(j + 1) * fc, :])
                t_tile = io_pool.tile([P, fc], F32)
                nc.sync.dma_start(out=t_tile, in_=tgt[:, j * fc:(j + 1) * fc])

                e_tile = work.tile([P, fc, C], F32)
                nc.scalar.activation(
                    out=e_tile, in_=p_tile, func=mybir.ActivationFunctionType.Exp
                )
                s_tile = work.tile([P, fc], F32)
                nc.vector.tensor_reduce(
                    out=s_tile, in_=e_tile, axis=mybir.AxisListType.X,
                    op=mybir.AluOpType.add,
                )
                lse_tile = work.tile([P, fc], F32)
                nc.scalar.activation(
                    out=lse_tile, in_=s_tile, func=mybir.ActivationFunctionType.Ln,
                    accum_out=lse_acc[:, j:j + 1],
                )

                for c in range(C):
                    mask = work.tile([P, fc], F32)
                    nc.vector.tensor_scalar(
                        out=mask, in0=t_tile, scalar1=float(c), scalar2=None,
                        op0=mybir.AluOpType.is_equal,
                    )
                    prod = work.tile([P, fc], F32)
                    nc.vector.tensor_tensor(
                        out=prod, in0=p_tile[:, :, c], in1=mask,
                        op=mybir.AluOpType.mult,
                    )
                    k = j * C + c
                    nc.vector.tensor_reduce(
                        out=pt_acc[:, k:k + 1], in_=prod,
                        axis=mybir.AxisListType.X, op=mybir.AluOpType.add,
                    )

            nc.vector.tensor_reduce(
                out=res[:, 0:1], in_=lse_acc, axis=mybir.AxisListType.X,
                op=mybir.AluOpType.add,
            )
            nc.vector.tensor_reduce(
                out=res[:, 1:2], in_=pt_acc, axis=mybir.AxisListType.X,
                op=mybir.AluOpType.add,
            )
            nc.sync.dma_start(out=out, in_=res)
    if finalize:
        nc.finalize()
    return nc


_NC_CACHE = {}


def _get_nc(f_total=1024, nchunk=8):
    key = (f_total, nchunk)
    if key not in _NC_CACHE:
        _NC_CACHE[key] = build_nc(f_total, nchunk)
    return _NC_CACHE[key]


def prep_inputs(preds, targets):
    """Host-side shard prep: slice S=-1, interleave channels, split 8 ways."""
    p = np.asarray(preds)[:, -1]       # (4, 4, 512, 512) f32
    t = np.asarray(targets)[:, -1]     # (4, 512, 512) int
    # pixel-major, channel-last
    pi = np.ascontiguousarray(np.transpose(p, (0, 2, 3, 1)))
    pi = pi.reshape(N_CORES, P, -1, C)
    tf = t.astype(np.float32).reshape(N_CORES, P, -1)
    return [{"preds": pi[k], "tgt": tf[k]} for k in range(N_CORES)]


def reduce_outputs(results):
    total = 0.0
    for d in results:
        o = d["out"].astype(np.float64)
        total += float(o[:, 0].sum() - o[:, 1].sum())
    return np.float32(total / N_BATCH)


def kernel(preds, targets, _trace=False, _trace_kwargs=None):
    from concourse.bass_utils import run_bass_kernel_spmd

    in_maps = prep_inputs(preds, targets)
    f_total = in_maps[0]["preds"].shape[1]
    nc = _get_nc(f_total=f_total)
    r = run_bass_kernel_spmd(
        nc, in_maps, core_ids=list(range(N_CORES)),
        trace=_trace, **(_trace_kwargs or {}),
    )
    kernel.last_run = r
    return reduce_outputs(r.results)


kernel.last_run = None


# revision 6
# speedup vs baseline: 1.8678x; 1.8678x over previous
import numpy as np
import ml_dtypes

import concourse.bacc as bacc
import concourse.tile as tile
from concourse import mybir

# Problem: NIMSCrossEntropyLoss
#   preds (4, 4, 4, 512, 512) f32, targets (4, 4, 512, 512) int32
#   Only the S=-1 slice contributes:
#   loss = [sum_pixels logsumexp_c(p) - sum_pixels p[target]] / N_BATCH
# Shard the 4*512*512 = 1048576 pixels over 8 cores:
#   131072 pixels/core as [128 partitions, 1024 free] channel planes (bf16).

N_CORES = 8
P = 128           # partitions
C = 4             # classes
N_BATCH = 4       # reference divides by this
F = 1024          # pixels per partition per core

BF16 = mybir.dt.bfloat16
F32 = mybir.dt.float32

_PATCHED = False


def _patch_act_tables():
    """Force exp+ln into the combined ACT table so only one table load is
    emitted (greedy per-function set choice otherwise alternates sets)."""
    global _PATCHED
    if _PATCHED:
        return
    import concourse.hw_specs as hw_specs
    real = hw_specs.get_activation_tables
    Exp = mybir.ActivationFunctionType.Exp
    Ln = mybir.ActivationFunctionType.Ln

    def patched(arch):
        out = {}
        for name, fns in dict(real(arch)).items():
            if name != "natural_log_exp_and_others":
                fns = fns - {Exp, Ln}
            out[name] = fns
        return out

    bacc.get_activation_tables = patched
    _PATCHED = True


def build_nc(f=F, finalize=True):
    """One core's shard: pA = channels 0,1 planes [P, 2f]; pB = channels 2,3;
    tgt [P, f]; out [P, 5] f32 = per-partition sums (pt_c0..3, lse)."""
    _patch_act_tables()
    nc = bacc.Bacc("TRN2", target_bir_lowering=False, debug=False)
    pA = nc.dram_tensor("pA", (P, 2 * f), BF16, kind="ExternalInput").ap()
    pB = nc.dram_tensor("pB", (P, 2 * f), BF16, kind="ExternalInput").ap()
    tgt = nc.dram_tensor("tgt", (P, f), BF16, kind="ExternalInput").ap()
    out = nc.dram_tensor("out", (P, 5), F32, kind="ExternalOutput").ap()

    Exp = mybir.ActivationFunctionType.Exp
    Ln = mybir.ActivationFunctionType.Ln

    with tile.TileContext(nc) as tc:
        with tc.tile_pool(name="w", bufs=1) as w:
            pa = w.tile([P, 2 * f], BF16)
            tt = w.tile([P, f], BF16)
            pb = w.tile([P, 2 * f], BF16)
            nc.sync.dma_start(out=pa, in_=pA)
            nc.sync.dma_start(out=tt, in_=tgt)
            nc.sync.dma_start(out=pb, in_=pB)

            res = w.tile([P, 5], F32)

            eA = w.tile([P, 2 * f], BF16)
            nc.scalar.activation(out=eA, in_=pa, func=Exp)
            eB = w.tile([P, 2 * f], BF16)
            nc.scalar.activation(out=eB, in_=pb, func=Exp)

            # p_t path: (t == c) * p_c with fused accumulation, 1 instr/channel
            scr = w.tile([P, 4 * f], BF16)
            planes = [(pa, 0), (pa, 1), (pb, 0), (pb, 1)]
            for c in range(C):
                src, half = planes[c]
                nc.vector.scalar_tensor_tensor(
                    out=scr[:, c * f:(c + 1) * f], in0=tt, scalar=float(c),
                    in1=src[:, half * f:(half + 1) * f],
                    op0=mybir.AluOpType.is_equal, op1=mybir.AluOpType.mult,
                    accum_out=res[:, c:c + 1],
                )

            s01 = w.tile([P, f], BF16)
            nc.vector.tensor_tensor(out=s01, in0=eA[:, 0:f], in1=eA[:, f:2 * f],
                                    op=mybir.AluOpType.add)
            s23 = w.tile([P, f], BF16)
            nc.vector.tensor_tensor(out=s23, in0=eB[:, 0:f], in1=eB[:, f:2 * f],
                                    op=mybir.AluOpType.add)
            s = w.tile([P, f], BF16)
            nc.vector.tensor_tensor(out=s, in0=s01, in1=s23,
                                    op=mybir.AluOpType.add)

            lnout = w.tile([P, f], BF16)
            nc.scalar.activation(out=lnout, in_=s, func=Ln,
                                 accum_out=res[:, 4:5])

            nc.sync.dma_start(out=out, in_=res)
    if finalize:
        nc.finalize()
    return nc


_NC_CACHE = {}


def _get_nc(f=F):
    if f not in _NC_CACHE:
        _NC_CACHE[f] = build_nc(f)
    return _NC_CACHE[f]


def prep_inputs(preds, targets):
    """Host-side shard prep: S=-1 slice, channel-major planes, 8-way split."""
    p = np.asarray(preds)[:, -1]       # (N=4, C=4, 512, 512) f32
    t = np.asarray(targets)[:, -1]     # (4, 512, 512) int
    arr = np.transpose(p, (1, 0, 2, 3)).reshape(C, N_CORES, P, -1)
    arr = arr.astype(ml_dtypes.bfloat16)
    tf = t.reshape(N_CORES, P, -1).astype(ml_dtypes.bfloat16)
    maps = []
    for k in range(N_CORES):
        a = np.ascontiguousarray(arr[0:2, k].transpose(1, 0, 2)).reshape(P, -1)
        b = np.ascontiguousarray(arr[2:4, k].transpose(1, 0, 2)).reshape(P, -1)
        maps.append({"pA": a, "pB": b, "tgt": tf[k]})
    return maps


def reduce_outputs(results):
    total = 0.0
    for d in results:
        o = d["out"].astype(np.float64)
        total += float(o[:, 4].sum() - o[:, 0:4].sum())
    return np.float32(total / N_BATCH)


def kernel(preds, targets, _trace=False, _trace_kwargs=None):
    from concourse.bass_utils import run_bass_kernel_spmd

    in_maps = prep_inputs(preds, targets)
    f = in_maps[0]["tgt"].shape[1]
    nc = _get_nc(f=f)
    r = run_bass_kernel_spmd(
        nc, in_maps, core_ids=list(range(N_CORES)),
        trace=_trace, **(_trace_kwargs or {}),
    )
    kernel.last_run = r
    return reduce_outputs(r.results)


kernel.last_run = None
